# revision 1
# baseline (speedup 1.0000x reference)
"""Trainium2 Bass kernel for NEAT-style fixed-topology network evaluation.

v3: node-sharded data layout (vs batch-sharded baseline). Each of the 8 cores
evaluates 256 nodes per layer (2 tiles of 128) for the FULL batch of 1024,
then a per-layer AllGather concatenates the 8 shards straight into every
core's private DRAM value table. Layer 4 (only the last 256 nodes are
observed) is computed redundantly on every core, so no final collective.

Why: SWDGE gather descriptor generation on the Pool engine costs ~8.5ns per
index regardless of element size (measured 867us Pool-busy in the
batch-sharded baseline, 91% of its 948us runtime). Widening each gathered
row to the full 1024-batch (2KB) cuts per-core descriptors 8x: Pool drops to
~130us, paid for with 4 AllGathers (~35us each for 4MB).

The node shard is pure data (per-core gather indices / weights / biases), so
the SPMD program is identical on all cores; no per-core addresses needed.
"""

import os
import sys

for _p in ("/opt/trn_rl_repo", "/root/.axon_site/_ro/trn_rl_repo"):
    if os.path.isdir(_p) and _p not in sys.path:
        sys.path.insert(0, _p)

import numpy as np
import ml_dtypes

BF16 = ml_dtypes.bfloat16

B = 1024
N_IN = 1024
L = 5
NPL = 2048
FANIN = 16
N_OUT = 256
NCORES = 8
HI = [N_IN + l * NPL for l in range(L)]  # [1024, 3072, 5120, 7168, 9216]
NT = HI[4]

GCH = 1024  # idxs per dma_gather instruction (hw cap)
# per-core work: layers 1..3 -> my 2 tiles (2c, 2c+1); layer 4 -> tiles 14,15
# on every core (output tiles, computed redundantly).
N_MYTILES = 8  # 2 per layer for l=0..3 ... (layer0 dense) ; gather tiles: l1..3 2 each + l4 2
GTILES_PER_CORE = 6  # (l,2c),(l,2c+1) for l in 1..3; layer 4 is a 512-idx chunk
N_IDX = GTILES_PER_CORE * NPL + 512  # 12800 idxs per core
IDX_SB_COLS = N_IDX // 16  # 800

_PROG_CACHE = {}


def _build_program():
    import concourse.mybir as mybir
    import concourse.tile as tile
    from concourse import bacc

    dt = mybir.dt
    AF = mybir.ActivationFunctionType

    nc = bacc.Bacc(None, target_bir_lowering=False)

    tbl08 = nc.declare_dram_parameter("tbl08", [N_IN, B], dt.float8e4, isOutput=False)
    w0 = nc.declare_dram_parameter("w0", [N_IN, 256], dt.bfloat16, isOutput=False)
    idx = nc.declare_dram_parameter("idx", [128, IDX_SB_COLS], dt.int16, isOutput=False)
    wcols = nc.declare_dram_parameter("wcols", [128, 16 * GTILES_PER_CORE], dt.bfloat16, isOutput=False)
    bias = nc.declare_dram_parameter("bias", [128, 10], dt.float32, isOutput=False)
    ident = nc.declare_dram_parameter("ident", [128, 128], dt.bfloat16, isOutput=False)
    l32 = nc.declare_dram_parameter("l32", [4, 128, 32], dt.bfloat16, isOutput=False)
    out = nc.declare_dram_parameter("out", [32, B], dt.float32, isOutput=True)

    tbl = nc.dram_tensor("tbl", [NT, B], dt.float8e4)
    cc_in = nc.dram_tensor("cc_in", [256, B], dt.float8e4)

    with tile.TileContext(nc) as tc:
        with (
            tc.tile_pool(name="const", bufs=1) as constp,
            tc.tile_pool(name="g", bufs=3) as gp,
            tc.tile_pool(name="wd", bufs=2) as wdp,
            tc.tile_pool(name="acts", bufs=2) as actp,
            tc.tile_pool(name="psum", bufs=4, space="PSUM") as psump,
        ):
            # ---- preload constants ----
            idx_sb = constp.tile([128, IDX_SB_COLS], dt.int16)
            nc.sync.dma_start(out=idx_sb[:], in_=idx[:])
            wcols_sb = constp.tile([128, 16 * GTILES_PER_CORE], dt.bfloat16)
            nc.sync.dma_start(out=wcols_sb[:], in_=wcols[:])
            bias_sb = constp.tile([128, 10], dt.float32)
            nc.sync.dma_start(out=bias_sb[:], in_=bias[:])
            ident_sb = constp.tile([128, 128], dt.bfloat16)
            nc.sync.dma_start(out=ident_sb[:], in_=ident[:])
            inp_sb = constp.tile([128, 8, B], dt.float8e4)
            nc.sync.dma_start(
                out=inp_sb[:], in_=tbl08.rearrange("(k p) b -> p k b", p=128)
            )
            l32_sb = constp.tile([128, 4, 32], dt.bfloat16)
            nc.sync.dma_start(out=l32_sb[:], in_=l32.rearrange("j p m -> p j m"))
            w0_sb = constp.tile([128, 8, 256], dt.bfloat16)
            nc.sync.dma_start(
                out=w0_sb[:], in_=w0.rearrange("(k p) n -> p k n", p=128)
            )
            # seed table rows [0, N_IN) with the full inputs (fp8)
            nc.sync.dma_start(out=tbl[0:N_IN, :], in_=tbl08[:])

            def compute_tile(g, wd_cols, bias_col, act_fn, dst_sb, dst_q):
                """16 diag matmuls (2 batch chunks) + activation into dst."""
                wd = wdp.tile([128, 16, 128], dt.bfloat16, tag="wd")
                nc.vector.tensor_tensor(
                    out=wd[:],
                    in0=ident_sb[:].unsqueeze(1).broadcast_to([128, 16, 128]),
                    in1=wcols_sb[:, wd_cols : wd_cols + 16]
                    .unsqueeze(2)
                    .broadcast_to([128, 16, 128]),
                    op=mybir.AluOpType.mult,
                )
                for cch in range(2):
                    ps = psump.tile([128, 512], dt.float32)
                    for f in range(16):
                        nc.tensor.matmul(
                            out=ps[:],
                            lhsT=wd[:, f, :],
                            rhs=g[:, f, 512 * cch : 512 * (cch + 1)],
                            start=(f == 0),
                            stop=(f == 15),
                        )
                    nc.scalar.activation(
                        out=dst_sb[:, dst_q, 512 * cch : 512 * (cch + 1)],
                        in_=ps[:],
                        func=act_fn,
                        bias=bias_sb[:, bias_col : bias_col + 1],
                    )

            # ---- layer 0: dense matmul, my 256 nodes ----
            act_buf = actp.tile([128, 2, B], dt.bfloat16)
            for q in range(2):
                for cch in range(2):
                    ps = psump.tile([128, 512], dt.float32)
                    for k in range(8):
                        nc.tensor.matmul(
                            out=ps[:],
                            lhsT=w0_sb[:, k, 128 * q : 128 * (q + 1)],
                            rhs=inp_sb[:, k, 512 * cch : 512 * (cch + 1)],
                            start=(k == 0),
                            stop=(k == 7),
                        )
                    nc.scalar.activation(
                        out=act_buf[:, q, 512 * cch : 512 * (cch + 1)],
                        in_=ps[:],
                        func=AF.Tanh,
                        bias=bias_sb[:, q : q + 1],
                    )
            # publish my 256 rows (fp8) -> cc_in, AllGather into tbl rows [N_IN, ...)
            act8 = actp.tile([128, 2, B], dt.float8e4, tag="act8")
            nc.vector.tensor_copy(out=act8[:], in_=act_buf[:])
            nc.sync.dma_start(
                out=cc_in.rearrange("(q p) b -> p q b", p=128), in_=act8[:]
            )
            nc.gpsimd.collective_compute(
                "AllGather",
                mybir.AluOpType.bypass,
                replica_groups=[list(range(NCORES))],
                ins=[cc_in.ap().opt()],
                outs=[tbl[HI[0] : HI[0] + NPL, :].opt()],
            )

            # ---- layers 1..3 ----
            icol = 0
            gi = 0
            for l in (1, 2, 3):
                act_buf = actp.tile([128, 2, B], dt.bfloat16)
                for q in range(2):
                    g = gp.tile([128, 16, B], dt.float8e4, tag="g")
                    for h in range(2):
                        nc.gpsimd.dma_gather(
                            out_ap=g[:, 8 * h : 8 * h + 8, :],
                            in_ap=tbl[0 : HI[l], :],
                            idxs_ap=idx_sb[:, icol : icol + GCH // 16],
                            num_idxs=GCH,
                            num_idxs_reg=GCH,
                            elem_size=B,
                        )
                        icol += GCH // 16
                    compute_tile(g, 16 * gi, 2 + 2 * (l - 1) + q, AF.Tanh, act_buf, q)
                    gi += 1
                act8 = actp.tile([128, 2, B], dt.float8e4, tag="act8")
                nc.vector.tensor_copy(out=act8[:], in_=act_buf[:])
                nc.sync.dma_start(
                    out=cc_in.rearrange("(q p) b -> p q b", p=128), in_=act8[:]
                )
                nc.gpsimd.collective_compute(
                    "AllGather",
                    mybir.AluOpType.bypass,
                    replica_groups=[list(range(NCORES))],
                    ins=[cc_in.ap().opt()],
                    outs=[tbl[HI[l] : HI[l] + NPL, :].opt()],
                )

            # ---- layer 4: my 32 output nodes (512 edges, one gather) ----
            # g4[p, j, b] = value of edge (node p%32, fanin 4*(p//32)+j);
            # lhsT32[j][p, m] scatters it to psum row m with its weight.
            out_sb = constp.tile([32, B], dt.float32)
            g4 = gp.tile([128, 4, B], dt.float8e4, tag="g4")
            nc.gpsimd.dma_gather(
                out_ap=g4[:],
                in_ap=tbl[0 : HI[4], :],
                idxs_ap=idx_sb[:, icol : icol + 32],
                num_idxs=512,
                num_idxs_reg=512,
                elem_size=B,
            )
            for cch in range(2):
                ps4 = psump.tile([32, 512], dt.float32)
                for j in range(4):
                    nc.tensor.matmul(
                        out=ps4[:],
                        lhsT=l32_sb[:, j, :],
                        rhs=g4[:, j, 512 * cch : 512 * (cch + 1)],
                        start=(j == 0),
                        stop=(j == 3),
                    )
                nc.scalar.activation(
                    out=out_sb[:, 512 * cch : 512 * (cch + 1)],
                    in_=ps4[:],
                    func=AF.Sigmoid,
                    bias=bias_sb[0:32, 8:9],
                )
            nc.sync.dma_start(out=out[:], in_=out_sb[:])

    nc.finalize()
    return nc


def get_program():
    if "nc" not in _PROG_CACHE:
        _PROG_CACHE["nc"] = _build_program()
    return _PROG_CACHE["nc"]


def _host_inputs(inputs, edge_src, edge_w, biases):
    """Build per-core input maps. Core c owns nodes [256c, 256c+256) of layers
    0..3; every core owns layer-4 tiles 14, 15."""
    inputs = np.asarray(inputs, dtype=np.float32)
    edge_src = np.asarray(edge_src, dtype=np.int64)
    edge_w = np.asarray(edge_w, dtype=np.float32)
    biases = np.asarray(biases, dtype=np.float32)

    tbl08 = np.ascontiguousarray(inputs.T).astype(ml_dtypes.float8_e4m3)
    ident = np.eye(128, dtype=BF16)

    in_maps = []
    for c in range(NCORES):
        # layer-0 dense weights for my 256 nodes
        w0 = np.zeros((N_IN, 256), dtype=np.float32)
        sl = slice(256 * c, 256 * c + 256)
        np.add.at(
            w0,
            (edge_src[0][sl].ravel(), np.repeat(np.arange(256), FANIN)),
            edge_w[0][sl].ravel(),
        )
        # gather tiles for this core: (l, global tile t)
        gtiles = [(l, 2 * c + q) for l in (1, 2, 3) for q in (0, 1)]
        idx_parts = []
        wcol_parts = []
        for (l, t) in gtiles:
            es = edge_src[l][128 * t : 128 * (t + 1)]  # [128, 16]
            ew = edge_w[l][128 * t : 128 * (t + 1)]
            for h in range(2):
                idx_parts.append(
                    es[:, 8 * h : 8 * h + 8].T.reshape(-1).astype(np.int16)
                )
            wcol_parts.append(ew.astype(BF16))
        # layer-4: my 32 output nodes; idx position i=(j*128+p) -> edge
        # (node p%32, fanin 4*(p//32)+j)
        es4 = edge_src[4][1792 + 32 * c : 1792 + 32 * c + 32]  # [32, 16]
        ew4 = edge_w[4][1792 + 32 * c : 1792 + 32 * c + 32]
        idx4 = np.empty(512, dtype=np.int16)
        l32 = np.zeros((4, 128, 32), dtype=np.float32)
        for i in range(512):
            p, j = i % 128, i // 128
            n, f = p % 32, 4 * (p // 32) + j
            idx4[i] = es4[n, f]
            l32[j, p, n] = ew4[n, f]
        idx_parts.append(idx4)
        idx_sb = np.empty((16, IDX_SB_COLS), dtype=np.int16)
        col = 0
        for part in idx_parts:
            ncol = part.size // 16
            idx_sb[:, col : col + ncol] = part.reshape(ncol, 16).T
            col += ncol
        idx_sb = np.tile(idx_sb, (8, 1))
        wcols = np.concatenate(wcol_parts, axis=1)

        # bias columns: l0 q0,q1 | l1 q0,q1 | l2 | l3 | l4 t14,t15
        bias_arr = np.zeros((128, 10), dtype=np.float32)
        for li, l in enumerate((0, 1, 2, 3)):
            for q in (0, 1):
                t = 2 * c + q
                bias_arr[:, 2 * li + q] = biases[l][128 * t : 128 * (t + 1)]
        bias_arr[0:32, 8] = biases[4][1792 + 32 * c : 1792 + 32 * c + 32]

        in_maps.append(
            {
                "tbl08": tbl08,
                "w0": w0.astype(BF16),
                "idx": idx_sb,
                "wcols": wcols,
                "bias": bias_arr,
                "ident": ident,
                "l32": l32.astype(BF16),
            }
        )
    return in_maps


def kernel(inputs, edge_src, edge_w, biases):
    from concourse.bass_utils import run_bass_kernel_spmd

    nc = get_program()
    in_maps = _host_inputs(inputs, edge_src, edge_w, biases)
    res = run_bass_kernel_spmd(nc, in_maps, core_ids=list(range(NCORES)))
    return np.concatenate(
        [np.asarray(res.results[c]["out"]) for c in range(NCORES)], axis=0
    ).T.astype(np.float32)

